# revision 1
# baseline (speedup 1.0000x reference)
"""Multi-head causal self-attention (B=2, S=2048, D=2048, H=16) on 8 trn2
NeuronCores.

Sharding: tensor-parallel over heads. Core c owns heads {2c, 2c+1}:
  - QKV projection for its 2 heads (contraction over the full d_model),
  - causal attention for its 2 heads,
  - partial output projection  O_c = A_c @ W_out[:, c*256:(c+1)*256].T
Host sums the 8 partial outputs (the "all-reduce after out_proj" of the
TP scheme, done on host since the full output is assembled there anyway).

All on-device compute is laid out "feature-major" (transposed) so no
transposes are ever needed:
  - x is shipped as xT [D, B*S]
  - Q^T, K^T per head as [Dh=128, S];  V token-major as [S, Dh] tiles
  - scores are built transposed: S^T[k, q] = (K Q^T)[k, q]
  - softmax without max-subtraction (scores are O(+-5)), with the
    normalizer computed by a ones-row matmul over partition (key) dim and
    applied via a rank-1 PE broadcast.
  - attention output lands as A^T [Dh, S]; out-proj consumes it directly.

Matmuls run as float32r (full PE rate at free-dim >= 256, fp32 storage).
"""

import math

import ml_dtypes
import numpy as np

import concourse.bass as bass
import concourse.tile as tile
from concourse import bacc, mybir
from concourse.bass_utils import run_bass_kernel_spmd

F32 = mybir.dt.float32
F32R = mybir.dt.float32r
BF16 = mybir.dt.bfloat16

N_CORES = 8


class Cfg:
    def __init__(self, B=2, S=2048, D=2048, n_heads=16):
        self.B = B
        self.S = S
        self.D = D
        self.n_heads = n_heads
        self.Dh = 128
        self.DHT = n_heads * self.Dh       # W_qkv section stride (q/k/v)
        self.HPC = n_heads // N_CORES      # heads per core (2)
        self.QC = 512                      # token chunk (matmul free dim)
        self.KT = D // 128                 # k-tiles over d_model
        self.NCH = S // self.QC            # token chunks per batch
        assert self.HPC == 2 and D % 128 == 0 and S % self.QC == 0


def build_kernel(cfg: Cfg):
    """Build the SPMD single-core program. Returns compiled nc."""
    B, S, D, QC, KT, NCH = cfg.B, cfg.S, cfg.D, cfg.QC, cfg.KT, cfg.NCH
    Dh = cfg.Dh
    NQT = QC // 128                      # 128-token subtiles per chunk
    inv_sqrt_dh = 1.0 / math.sqrt(Dh)

    nc = bacc.Bacc("TRN2", target_bir_lowering=False, debug=False,
                   num_devices=N_CORES)

    xT = nc.dram_tensor("xT", [D, B * S], F32R, kind="ExternalInput").ap()
    wqkvT = nc.dram_tensor("wqkvT", [D, 768], F32R, kind="ExternalInput").ap()
    woutT = nc.dram_tensor("woutT", [256, D], F32R, kind="ExternalInput").ap()
    masks = nc.dram_tensor("masks", [128, NQT * QC], BF16,
                           kind="ExternalInput").ap()
    ones_col = nc.dram_tensor("ones_col", [128, 1], BF16,
                              kind="ExternalInput").ap()
    ones_row = nc.dram_tensor("ones_row", [1, 128], F32R,
                              kind="ExternalInput").ap()
    outT = nc.dram_tensor("outT", [D, B * S], F32, kind="ExternalOutput").ap()


    with tile.TileContext(nc) as tc:
        with (
            tc.tile_pool(name="wpool", bufs=1) as wpool,
            tc.tile_pool(name="xpool", bufs=5) as xpool,
            tc.tile_pool(name="qkvpool", bufs=1) as qkvpool,
            tc.tile_pool(name="apool", bufs=1) as apool,
            tc.tile_pool(name="ppool", bufs=7) as ppool,
            tc.tile_pool(name="opool", bufs=3) as opool,
            tc.tile_pool(name="smallpool", bufs=2) as smallpool,
            tc.tile_pool(name="pspool", bufs=4, space="PSUM") as pspool,
            tc.tile_pool(name="attnps", bufs=2, space="PSUM") as attnps,
            tc.tile_pool(name="rps", bufs=2, space="PSUM") as rps,
        ):
            # ---- static weights / constants ----
            w_tiles = []
            for k in range(KT):
                t = wpool.tile([128, 768], F32R, tag=f"w{k}", name=f"w{k}")
                nc.sync.dma_start(t[:], wqkvT[k * 128:(k + 1) * 128, :])
                w_tiles.append(t)
            wo_tiles = []
            for hh in range(2):
                t = wpool.tile([128, D], F32R, tag=f"wo{hh}", name=f"wo{hh}")
                nc.sync.dma_start(t[:], woutT[hh * 128:(hh + 1) * 128, :])
                wo_tiles.append(t)
            mask_t = wpool.tile([128, NQT * QC], BF16, tag="mask", name="mask")
            nc.sync.dma_start(mask_t[:], masks[:])
            onec_t = wpool.tile([128, 1], BF16, tag="onec", name="onec")
            nc.sync.dma_start(onec_t[:], ones_col[:])
            oner_t = wpool.tile([1, 128], F32R, tag="oner", name="oner")
            nc.sync.dma_start(oner_t[:], ones_row[:])

            for b in range(B):
                # ---- persistent per-batch QKV / A tiles ----
                # comps: 0=Q_h0 1=K_h0 2=Q_h1 3=K_h1 (dh-major [128, S])
                qk_sb = [qkvpool.tile([128, S], F32R, tag=f"qk{c}", name=f"qk{c}")
                         for c in range(4)]
                # V token-major: tile per 128 tokens, [128, 256] (2 heads)
                v_sb = [qkvpool.tile([128, 256], BF16, tag=f"v{t}", name=f"v{t}")
                        for t in range(S // 128)]
                # A^T per head [128, S]
                a_sb = [apool.tile([128, S], F32R, tag=f"a{h}", name=f"a{h}")
                        for h in range(2)]

                # ======== Phase A: QKV projection for this batch ========
                HKT = KT // 4
                for j in range(NCH):
                    col0 = b * S + j * QC
                    # four batched DMAs per chunk: [128, HKT*QC] quarters
                    # with the d_model k-tiles laid out along the free dim
                    halves = []
                    for hh in range(4):
                        t = xpool.tile([128, HKT * QC], F32R, tag="xt",
                                       name="xt")
                        src = xT[hh * HKT * 128:(hh + 1) * HKT * 128,
                                 col0:col0 + QC]
                        nc.sync.dma_start(
                            t[:].rearrange("p (k c) -> p k c", k=HKT),
                            src.rearrange("(k p) c -> p k c", p=128))
                        halves.append(t)

                    def xt_sl(k, f0, f1):
                        t = halves[k // HKT]
                        kk = k % HKT
                        return t[:, kk * QC + f0: kk * QC + f1]

                    # Q^T / K^T for both heads (copies on ScalarE: idle in
                    # this phase, keeps DVE free)
                    for c in range(4):
                        ps = pspool.tile([128, QC], F32, tag="ps", name="ps")
                        for k in range(KT):
                            nc.tensor.matmul(
                                ps[:],
                                (w_tiles[k][:, c * 128:(c + 1) * 128]),
                                (xt_sl(k, 0, QC)),
                                start=(k == 0), stop=(k == KT - 1))
                        nc.scalar.copy(
                            qk_sb[c][:, j * QC:(j + 1) * QC], ps[:])
                    # V token-major (both heads side by side)
                    for sub in range(NQT):
                        ps = pspool.tile([128, 256], F32, tag="ps", name="ps")
                        for k in range(KT):
                            nc.tensor.matmul(
                                ps[:],
                                (xt_sl(k, sub * 128, (sub + 1) * 128)),
                                (w_tiles[k][:, 512:768]),
                                start=(k == 0), stop=(k == KT - 1))
                        nc.scalar.copy(v_sb[j * NQT + sub][:], ps[:])

                # ======== Phase B+C: attention + out-proj per chunk ======
                # Normalization is software-pipelined one block behind so
                # the (slow) reciprocal never sits on the PE's in-order
                # path: block k's rank-1 broadcast + final mul are emitted
                # after block k+1's matmuls.

                def emit_attn_block(j, h):
                    # attnV/r matmuls lag the scores by SKEW k-tiles so the
                    # exp -> mask chain latency stays off the PE's in-order
                    # path.
                    SKEW = 2
                    n_kt = (j + 1) * QC // 128
                    qT = qk_sb[2 * h]
                    kTl = qk_sb[2 * h + 1]
                    attn = attnps.tile([128, QC], F32, tag="attn",
                                       name="attn")
                    r = rps.tile([1, QC], F32, tag="r", name="r")
                    p_tiles = {}

                    def emit_scores(kt):
                        rel = kt * 128 - j * QC
                        # causal trim: queries below the diagonal block's
                        # start contribute nothing. fp32r needs N>=256 for
                        # full rate, bf16 consumers can trim all the way.
                        f_sc = min(max(rel, 0), QC - 256)
                        f_av = max(rel, 0)
                        s_ps = pspool.tile([128, QC], F32, tag="ps",
                                           name="ps")
                        nc.tensor.matmul(
                            s_ps[:, f_sc:],
                            kTl[:, kt * 128:(kt + 1) * 128],
                            qT[:, j * QC + f_sc:(j + 1) * QC],
                            start=True, stop=True)
                        p_sb = ppool.tile([128, QC], BF16, tag="p", name="p")
                        nc.scalar.activation(
                            p_sb[:, f_av:], s_ps[:, f_av:],
                            mybir.ActivationFunctionType.Exp,
                            scale=inv_sqrt_dh)
                        if rel >= 0:
                            # diagonal block: zero the k > q half
                            ridx = rel // 128
                            nc.vector.tensor_mul(
                                p_sb[:, f_av:], p_sb[:, f_av:],
                                mask_t[:, ridx * QC + f_av:(ridx + 1) * QC])
                        p_tiles[kt] = (p_sb, f_av)

                    def emit_av(kt):
                        p_sb, f_av = p_tiles.pop(kt)
                        nc.tensor.matmul(
                            attn[:, f_av:],
                            v_sb[kt][:, h * 128:(h + 1) * 128],
                            p_sb[:, f_av:],
                            start=(kt == 0), stop=(kt == n_kt - 1))
                        nc.tensor.matmul(
                            r[:, f_av:], onec_t[:], p_sb[:, f_av:],
                            start=(kt == 0), stop=(kt == n_kt - 1))

                    for kt in range(n_kt):
                        emit_scores(kt)
                        if kt >= SKEW:
                            emit_av(kt - SKEW)
                    for kt in range(max(0, n_kt - SKEW), n_kt):
                        emit_av(kt)
                    # launch the reciprocal now (DVE), consumed one block
                    # later by the rank-1 broadcast
                    recip = smallpool.tile([1, QC], F32, tag="recip",
                                           name="recip")
                    nc.vector.reciprocal_approx_fast(recip[:], r[:])
                    recip_r = smallpool.tile([1, QC], F32R, tag="recipr",
                                             name="recipr")
                    nc.vector.tensor_copy(recip_r[:], recip[:])
                    return (j, h, attn, recip_r)

                def emit_finalize(blk):
                    j, h, attn, recip_r = blk
                    rb_ps = pspool.tile([128, QC], F32, tag="ps", name="ps")
                    nc.tensor.matmul(rb_ps[:], oner_t[:], recip_r[:],
                                     start=True, stop=True)
                    rb_sb = ppool.tile([128, QC], F32R, tag="p", name="p")
                    nc.vector.tensor_copy(rb_sb[:], rb_ps[:])
                    nc.vector.tensor_mul(
                        a_sb[h][:, j * QC:(j + 1) * QC], attn[:], rb_sb[:])

                def emit_outproj(j):
                    # partial over this core's 256 head-features; psum
                    # drains alternate DVE/ACT so the PE never waits on a
                    # slot, and the output DMAs ride the idle GpSimd SWDGE.
                    col0 = b * S + j * QC
                    for m in range(D // 128):
                        ps = pspool.tile([128, QC], F32, tag="ps", name="ps")
                        for h in range(2):
                            nc.tensor.matmul(
                                ps[:],
                                wo_tiles[h][:, m * 128:(m + 1) * 128],
                                a_sb[h][:, j * QC:(j + 1) * QC],
                                start=(h == 0), stop=(h == 1))
                        o_sb = opool.tile([128, QC], F32, tag="o", name="o")
                        nc.vector.tensor_copy(o_sb[:], ps[:])
                        nc.sync.dma_start(
                            outT[m * 128:(m + 1) * 128, col0:col0 + QC],
                            o_sb[:])

                pending = None
                for j in range(NCH):
                    for h in range(2):
                        blk = emit_attn_block(j, h)
                        if pending is not None:
                            emit_finalize(pending)
                            if pending[1] == 1:
                                emit_outproj(pending[0])
                        pending = blk
                emit_finalize(pending)
                emit_outproj(pending[0])

    nc.compile()
    return nc


def make_inputs(cfg: Cfg, x, W_qkv, W_out):
    """Host-side sharding: returns in_maps (list of 8 dicts)."""
    B, S, D = cfg.B, cfg.S, cfg.D
    Dh, QC, NQT = cfg.Dh, cfg.QC, cfg.QC // 128
    xTa = np.ascontiguousarray(
        x.reshape(B * S, D).T.astype(np.float32))          # [D, B*S]

    masks = np.zeros((128, NQT * QC), dtype=ml_dtypes.bfloat16)
    for ridx in range(NQT):
        rel = ridx * 128
        p = np.arange(128)[:, None]
        f = np.arange(QC)[None, :]
        masks[:, ridx * QC:(ridx + 1) * QC] = (p + rel <= f)
    ones_col = np.ones((128, 1), dtype=ml_dtypes.bfloat16)
    ones_row = np.ones((1, 128), dtype=np.float32)

    in_maps = []
    DHT = cfg.DHT
    for c in range(N_CORES):
        h0 = cfg.HPC * c
        wq = np.empty((D, 768), dtype=np.float32)          # [D, cols]
        for i, h in enumerate((h0, h0 + 1)):
            wq[:, (2 * i) * 128:(2 * i) * 128 + 128] = \
                W_qkv[0 * DHT + h * Dh: 0 * DHT + h * Dh + Dh, :].T   # Q_h
            wq[:, (2 * i + 1) * 128:(2 * i + 1) * 128 + 128] = \
                W_qkv[1 * DHT + h * Dh: 1 * DHT + h * Dh + Dh, :].T   # K_h
            wq[:, 512 + i * 128: 512 + (i + 1) * 128] = \
                W_qkv[2 * DHT + h * Dh: 2 * DHT + h * Dh + Dh, :].T   # V_h
        wo = np.ascontiguousarray(
            W_out[:, h0 * Dh:(h0 + cfg.HPC) * Dh].T.astype(np.float32))
        in_maps.append({
            "xT": xTa,
            "wqkvT": np.ascontiguousarray(wq),
            "woutT": wo,
            "masks": masks,
            "ones_col": ones_col,
            "ones_row": ones_row,
        })
    return in_maps


_CACHED = {}


def kernel(x, W_qkv, W_out, mask=None, **_ignored):
    cfg = Cfg(B=x.shape[0], S=x.shape[1], D=x.shape[2],
              n_heads=W_qkv.shape[0] // 384)
    key = (cfg.B, cfg.S, cfg.D)
    if key not in _CACHED:
        _CACHED[key] = build_kernel(cfg)
    nc = _CACHED[key]
    in_maps = make_inputs(cfg, np.asarray(x), np.asarray(W_qkv),
                          np.asarray(W_out))
    res = run_bass_kernel_spmd(nc, in_maps, list(range(N_CORES)))
    acc = res.results[0]["outT"].astype(np.float32)
    for c in range(1, N_CORES):
        acc = acc + res.results[c]["outT"]
    out = acc.T.reshape(cfg.B, cfg.S, cfg.D)
    return np.ascontiguousarray(out)



# revision 2
# speedup vs baseline: 1.0223x; 1.0223x over previous
"""Multi-head causal self-attention (B=2, S=2048, D=2048, H=16) on 8 trn2
NeuronCores.

Sharding: tensor-parallel over heads. Core c owns heads {2c, 2c+1}:
  - QKV projection for its 2 heads (contraction over the full d_model),
  - causal attention for its 2 heads,
  - partial output projection  O_c = A_c @ W_out[:, c*256:(c+1)*256].T
Host sums the 8 partial outputs (the "all-reduce after out_proj" of the
TP scheme, done on host since the full output is assembled there anyway).

All on-device compute is laid out "feature-major" (transposed) so no
transposes are ever needed:
  - x is shipped as xT [D, B*S]
  - Q^T, K^T per head as [Dh=128, S];  V token-major as [S, Dh] tiles
  - scores are built transposed: S^T[k, q] = (K Q^T)[k, q]
  - causal mask applied INSIDE the scores accumulation group: one extra
    128-row matmul adds -1e9 to the upper triangle of the diagonal
    128-block (stationary identity, moving triangle constant), so the
    exp -> AV chain never touches the vector engine.
  - softmax without max-subtraction (scores are O(+-5)); normalizer via
    ones-column matmul over the partition (key) dim, applied through a
    rank-1 PE broadcast.
  - attention output lands as A^T [Dh, S]; out-proj consumes it directly.

Pipeline: scores run SKEW=3 k-tiles ahead of attn*V so the PSUM->exp
latency stays off the PE's in-order path; normalization lags one block;
out-proj lags two blocks so its operands' DVE writes are long done.
Output stores ride the gpsimd queue so batch 1's input prefetch is not
stuck behind batch 0's stores on the sync queue.
"""

import math

import ml_dtypes
import numpy as np

import concourse.bass as bass
import concourse.tile as tile
from concourse import bacc, mybir
from concourse.bass_utils import run_bass_kernel_spmd

F32 = mybir.dt.float32
F32R = mybir.dt.float32r
BF16 = mybir.dt.bfloat16

N_CORES = 8


class Cfg:
    def __init__(self, B=2, S=2048, D=2048, n_heads=16):
        self.B = B
        self.S = S
        self.D = D
        self.n_heads = n_heads
        self.Dh = 128
        self.DHT = n_heads * self.Dh       # W_qkv section stride (q/k/v)
        self.HPC = n_heads // N_CORES      # heads per core (2)
        self.QC = 512                      # token chunk (matmul free dim)
        self.KT = D // 128                 # k-tiles over d_model
        self.NCH = S // self.QC            # token chunks per batch
        assert self.HPC == 2 and D % 128 == 0 and S % self.QC == 0


def build_kernel(cfg: Cfg):
    """Build the SPMD single-core program. Returns compiled nc."""
    B, S, D, QC, KT, NCH = cfg.B, cfg.S, cfg.D, cfg.QC, cfg.KT, cfg.NCH
    Dh = cfg.Dh
    NQT = QC // 128                      # 128-token subtiles per chunk
    inv_sqrt_dh = 1.0 / math.sqrt(Dh)

    nc = bacc.Bacc("TRN2", target_bir_lowering=False, debug=False,
                   num_devices=N_CORES)

    xT = nc.dram_tensor("xT", [D, B * S], F32R, kind="ExternalInput").ap()
    wqkvT = nc.dram_tensor("wqkvT", [D, 768], F32R, kind="ExternalInput").ap()
    woutT = nc.dram_tensor("woutT", [256, D], F32R, kind="ExternalInput").ap()
    ident = nc.dram_tensor("ident", [128, 128], BF16,
                           kind="ExternalInput").ap()
    tri = nc.dram_tensor("tri", [128, 128], BF16, kind="ExternalInput").ap()
    ones_col = nc.dram_tensor("ones_col", [128, 1], BF16,
                              kind="ExternalInput").ap()
    ones_row = nc.dram_tensor("ones_row", [1, 128], BF16,
                              kind="ExternalInput").ap()
    outT = nc.dram_tensor("outT", [D, B * S], F32, kind="ExternalOutput").ap()

    with tile.TileContext(nc) as tc:
        with (
            tc.tile_pool(name="wpool", bufs=1) as wpool,
            tc.tile_pool(name="xpool", bufs=5) as xpool,
            tc.tile_pool(name="qkvpool", bufs=1) as qkvpool,
            tc.tile_pool(name="apool", bufs=1) as apool,
            tc.tile_pool(name="ppool", bufs=7) as ppool,
            tc.tile_pool(name="opool", bufs=3) as opool,
            tc.tile_pool(name="smallpool", bufs=2) as smallpool,
            tc.tile_pool(name="pspool", bufs=4, space="PSUM") as pspool,
            tc.tile_pool(name="attnps", bufs=2, space="PSUM") as attnps,
            tc.tile_pool(name="rps", bufs=2, space="PSUM") as rps,
        ):
            HKT = KT // 4

            def load_x_chunk(b, j):
                """Four quarter DMAs for one 512-token chunk of x^T."""
                col0 = b * S + j * QC
                halves = []
                for hh in range(4):
                    t = xpool.tile([128, HKT * QC], F32R, tag="xt", name="xt")
                    src = xT[hh * HKT * 128:(hh + 1) * HKT * 128,
                             col0:col0 + QC]
                    nc.sync.dma_start(
                        t[:].rearrange("p (k c) -> p k c", k=HKT),
                        src.rearrange("(k p) c -> p k c", p=128))
                    halves.append(t)
                return halves

            # ---- static weights / constants, interleaved so the first
            # QKV matmuls (need w k-tile 0 + x chunk 0) start ASAP ----
            w_tiles = []
            for k in range(4):
                t = wpool.tile([128, 768], F32R, tag=f"w{k}", name=f"w{k}")
                nc.sync.dma_start(t[:], wqkvT[k * 128:(k + 1) * 128, :])
                w_tiles.append(t)
            first_chunk = load_x_chunk(0, 0)
            for k in range(4, KT):
                t = wpool.tile([128, 768], F32R, tag=f"w{k}", name=f"w{k}")
                nc.sync.dma_start(t[:], wqkvT[k * 128:(k + 1) * 128, :])
                w_tiles.append(t)
            onec_t = wpool.tile([128, 1], BF16, tag="onec", name="onec")
            nc.sync.dma_start(onec_t[:], ones_col[:])
            oner_t = wpool.tile([1, 128], BF16, tag="oner", name="oner")
            nc.sync.dma_start(oner_t[:], ones_row[:])
            ident_t = wpool.tile([128, 128], BF16, tag="ident", name="ident")
            nc.sync.dma_start(ident_t[:], ident[:])
            tri_t = wpool.tile([128, 128], BF16, tag="tri", name="tri")
            nc.sync.dma_start(tri_t[:], tri[:])
            wo_tiles = []
            for hh in range(2):
                t = wpool.tile([128, D], F32R, tag=f"wo{hh}", name=f"wo{hh}")
                nc.sync.dma_start(t[:], woutT[hh * 128:(hh + 1) * 128, :])
                wo_tiles.append(t)

            for b in range(B):
                # ---- persistent per-batch QKV / A tiles ----
                # comps: 0=Q_h0 1=K_h0 2=Q_h1 3=K_h1 (dh-major [128, S])
                qk_sb = [qkvpool.tile([128, S], F32R, tag=f"qk{c}", name=f"qk{c}")
                         for c in range(4)]
                # V token-major: tile per 128 tokens, [128, 256] (2 heads)
                v_sb = [qkvpool.tile([128, 256], BF16, tag=f"v{t}", name=f"v{t}")
                        for t in range(S // 128)]
                # A^T per head [128, S]
                a_sb = [apool.tile([128, S], F32R, tag=f"a{h}", name=f"a{h}")
                        for h in range(2)]

                # ======== Phase A: QKV projection for this batch ========
                for j in range(NCH):
                    if b == 0 and j == 0:
                        halves = first_chunk
                    else:
                        halves = load_x_chunk(b, j)

                    def xt_sl(k, f0, f1):
                        t = halves[k // HKT]
                        kk = k % HKT
                        return t[:, kk * QC + f0: kk * QC + f1]

                    # Q^T / K^T for both heads (copies on ScalarE: idle in
                    # this phase, keeps DVE free)
                    for c in range(4):
                        ps = pspool.tile([128, QC], F32, tag="ps", name="ps")
                        for k in range(KT):
                            nc.tensor.matmul(
                                ps[:],
                                (w_tiles[k][:, c * 128:(c + 1) * 128]),
                                (xt_sl(k, 0, QC)),
                                start=(k == 0), stop=(k == KT - 1))
                        nc.scalar.copy(
                            qk_sb[c][:, j * QC:(j + 1) * QC], ps[:])
                    # V token-major (both heads side by side)
                    for sub in range(NQT):
                        ps = pspool.tile([128, 256], F32, tag="ps", name="ps")
                        for k in range(KT):
                            nc.tensor.matmul(
                                ps[:],
                                (xt_sl(k, sub * 128, (sub + 1) * 128)),
                                (w_tiles[k][:, 512:768]),
                                start=(k == 0), stop=(k == KT - 1))
                        nc.scalar.copy(v_sb[j * NQT + sub][:], ps[:])

                # ======== Phase B+C: attention + out-proj per chunk ======
                # Normalization lags one block, out-proj two blocks, so the
                # DVE chain (reciprocal -> cast -> rank-1 drain -> mul)
                # never sits on the PE's in-order path.

                def emit_attn_block(j, h):
                    # attnV/r matmuls lag the scores by SKEW k-tiles so the
                    # PSUM -> exp latency stays off the PE's in-order path.
                    SKEW = 3
                    n_kt = (j + 1) * QC // 128
                    qT = qk_sb[2 * h]
                    kTl = qk_sb[2 * h + 1]
                    attn = attnps.tile([128, QC], F32, tag="attn",
                                       name="attn")
                    r = rps.tile([1, QC], F32, tag="r", name="r")
                    p_tiles = {}

                    def emit_scores(kt):
                        rel = kt * 128 - j * QC
                        # causal trim: queries below the diagonal block's
                        # start contribute nothing. fp32r needs N>=256 for
                        # full rate, bf16 consumers can trim all the way.
                        f_sc = min(max(rel, 0), QC - 256)
                        f_av = max(rel, 0)
                        diag = rel >= 0
                        s_ps = pspool.tile([128, QC], F32, tag="ps",
                                           name="ps")
                        nc.tensor.matmul(
                            s_ps[:, f_sc:],
                            kTl[:, kt * 128:(kt + 1) * 128],
                            qT[:, j * QC + f_sc:(j + 1) * QC],
                            start=True, stop=not diag)
                        if diag:
                            # diagonal block: add -1e9 to the k > q triangle
                            # (I.T @ tri) inside the accumulation group
                            nc.tensor.matmul(
                                s_ps[:, rel:rel + 128], ident_t[:], tri_t[:],
                                start=False, stop=True)
                        p_sb = ppool.tile([128, QC], BF16, tag="p", name="p")
                        nc.scalar.activation(
                            p_sb[:, f_av:], s_ps[:, f_av:],
                            mybir.ActivationFunctionType.Exp,
                            scale=inv_sqrt_dh)
                        p_tiles[kt] = (p_sb, f_av)

                    def emit_av(kt):
                        p_sb, f_av = p_tiles.pop(kt)
                        nc.tensor.matmul(
                            attn[:, f_av:],
                            v_sb[kt][:, h * 128:(h + 1) * 128],
                            p_sb[:, f_av:],
                            start=(kt == 0), stop=(kt == n_kt - 1))
                        nc.tensor.matmul(
                            r[:, f_av:], onec_t[:], p_sb[:, f_av:],
                            start=(kt == 0), stop=(kt == n_kt - 1))

                    for kt in range(n_kt):
                        emit_scores(kt)
                        if kt >= SKEW:
                            emit_av(kt - SKEW)
                    for kt in range(max(0, n_kt - SKEW), n_kt):
                        emit_av(kt)
                    # launch the reciprocal now (DVE), consumed one block
                    # later by the rank-1 broadcast
                    recip = smallpool.tile([1, QC], F32, tag="recip",
                                           name="recip")
                    nc.vector.reciprocal_approx_fast(recip[:], r[:])
                    recip_b = smallpool.tile([1, QC], BF16, tag="recipb",
                                             name="recipb")
                    nc.vector.tensor_copy(recip_b[:], recip[:])
                    return (j, h, attn, recip_b)

                def emit_finalize(blk):
                    j, h, attn, recip_b = blk
                    rb_ps = pspool.tile([128, QC], F32, tag="ps", name="ps")
                    nc.tensor.matmul(rb_ps[:], oner_t[:], recip_b[:],
                                     start=True, stop=True)
                    rb_sb = ppool.tile([128, QC], F32R, tag="p", name="p")
                    nc.vector.tensor_copy(rb_sb[:], rb_ps[:])
                    nc.vector.tensor_mul(
                        a_sb[h][:, j * QC:(j + 1) * QC], attn[:], rb_sb[:])

                def emit_outproj(j):
                    # partial over this core's 256 head-features; psum
                    # drains alternate DVE/ACT so neither queue backs up,
                    # and the output stores ride the idle gpsimd queue.
                    col0 = b * S + j * QC
                    for m in range(D // 128):
                        ps = pspool.tile([128, QC], F32, tag="ps", name="ps")
                        for h in range(2):
                            nc.tensor.matmul(
                                ps[:],
                                wo_tiles[h][:, m * 128:(m + 1) * 128],
                                a_sb[h][:, j * QC:(j + 1) * QC],
                                start=(h == 0), stop=(h == 1))
                        o_sb = opool.tile([128, QC], F32, tag="o", name="o")
                        if m % 2 == 0:
                            nc.vector.tensor_copy(o_sb[:], ps[:])
                        else:
                            nc.scalar.copy(o_sb[:], ps[:])
                        nc.gpsimd.dma_start(
                            outT[m * 128:(m + 1) * 128, col0:col0 + QC],
                            o_sb[:])

                blocks = []
                for j in range(NCH):
                    for h in range(2):
                        blocks.append(emit_attn_block(j, h))
                        n = len(blocks)
                        if n >= 2:
                            emit_finalize(blocks[n - 2])
                        if n >= 3 and blocks[n - 3][1] == 1:
                            emit_outproj(blocks[n - 3][0])
                emit_finalize(blocks[-1])
                emit_outproj(blocks[-1][0])

    nc.compile()
    return nc


def make_inputs(cfg: Cfg, x, W_qkv, W_out):
    """Host-side sharding: returns in_maps (list of 8 dicts)."""
    B, S, D = cfg.B, cfg.S, cfg.D
    Dh = cfg.Dh
    xTa = np.ascontiguousarray(
        x.reshape(B * S, D).T.astype(np.float32))          # [D, B*S]

    p = np.arange(128)[:, None]
    c = np.arange(128)[None, :]
    tri = np.where(p > c, -1e9, 0.0).astype(ml_dtypes.bfloat16)
    identm = np.eye(128, dtype=ml_dtypes.bfloat16)
    ones_col = np.ones((128, 1), dtype=ml_dtypes.bfloat16)
    ones_row = np.ones((1, 128), dtype=ml_dtypes.bfloat16)

    in_maps = []
    DHT = cfg.DHT
    for cidx in range(N_CORES):
        h0 = cfg.HPC * cidx
        wq = np.empty((D, 768), dtype=np.float32)          # [D, cols]
        for i, h in enumerate((h0, h0 + 1)):
            wq[:, (2 * i) * 128:(2 * i) * 128 + 128] = \
                W_qkv[0 * DHT + h * Dh: 0 * DHT + h * Dh + Dh, :].T   # Q_h
            wq[:, (2 * i + 1) * 128:(2 * i + 1) * 128 + 128] = \
                W_qkv[1 * DHT + h * Dh: 1 * DHT + h * Dh + Dh, :].T   # K_h
            wq[:, 512 + i * 128: 512 + (i + 1) * 128] = \
                W_qkv[2 * DHT + h * Dh: 2 * DHT + h * Dh + Dh, :].T   # V_h
        wo = np.ascontiguousarray(
            W_out[:, h0 * Dh:(h0 + cfg.HPC) * Dh].T.astype(np.float32))
        in_maps.append({
            "xT": xTa,
            "wqkvT": np.ascontiguousarray(wq),
            "woutT": wo,
            "ident": identm,
            "tri": tri,
            "ones_col": ones_col,
            "ones_row": ones_row,
        })
    return in_maps


_CACHED = {}


def kernel(x, W_qkv, W_out, mask=None, **_ignored):
    cfg = Cfg(B=x.shape[0], S=x.shape[1], D=x.shape[2],
              n_heads=W_qkv.shape[0] // 384)
    key = (cfg.B, cfg.S, cfg.D)
    if key not in _CACHED:
        _CACHED[key] = build_kernel(cfg)
    nc = _CACHED[key]
    in_maps = make_inputs(cfg, np.asarray(x), np.asarray(W_qkv),
                          np.asarray(W_out))
    res = run_bass_kernel_spmd(nc, in_maps, list(range(N_CORES)))
    acc = res.results[0]["outT"].astype(np.float32)
    for c in range(1, N_CORES):
        acc = acc + res.results[c]["outT"]
    out = acc.T.reshape(cfg.B, cfg.S, cfg.D)
    return np.ascontiguousarray(out)


# revision 3
# speedup vs baseline: 1.1792x; 1.1534x over previous
"""Multi-head causal self-attention (B=2, S=2048, D=2048, H=16) on 8 trn2
NeuronCores.

Sharding: tensor-parallel over heads. Core c owns heads {2c, 2c+1}:
  - QKV projection for its 2 heads (contraction over the full d_model),
  - causal attention for its 2 heads,
  - partial output projection  O_c = A_c @ W_out[:, c*256:(c+1)*256].T
Host sums the 8 partial outputs (the "all-reduce after out_proj" of the
TP scheme, done on host since the full output is assembled there anyway).

All on-device compute is laid out "feature-major" (transposed) so no
transposes are ever needed. Everything is bf16 except the PSUM (f32)
and the normalizer reciprocal: bf16 keeps every matmul at full PE rate
regardless of free-dim, enables fast weight load on all stationaries,
and halves DMA/SBUF traffic.

The attention inner loop is co-limited by the PE and the ACT engine
(exp costs (N+352)/1.2 ns per instruction), so scores are computed in
PAIRS of 128-key tiles into a two-bank [128, 1024] PSUM tile and a
single exp covers both: 573 ns/k-tile instead of 720. Three such score
tiles rotate (6 banks); attn and the normalizer each hold one bank and
are drained to SBUF at block end so bufs=1 suffices. The causal mask is
applied inside the scores accumulation group via one extra 128-row
matmul (adds -1e9 to the k > q triangle), keeping the exp -> AV chain
entirely off the vector engine.

Structure per batch (attention interleaved with QKV by chunk):
  for j: QKV(chunk j) -> attn(j,h0) -> fin(prev) -> attn(j,h1)
         -> fin(j,h0) -> outproj(j-1)
AV lags scores by 2 groups (4 k-tiles), normalization lags one block,
out-proj two blocks: every PE wait target is produced ~2.5 us earlier.
Output stores ride the gpsimd queue so input prefetch is never stuck
behind stores on the sync queue.
"""

import math

import ml_dtypes
import numpy as np

import concourse.bass as bass
import concourse.tile as tile
from concourse import bacc, mybir
from concourse.bass_utils import run_bass_kernel_spmd

F32 = mybir.dt.float32
F32R = mybir.dt.float32r
BF16 = mybir.dt.bfloat16

N_CORES = 8


class Cfg:
    def __init__(self, B=2, S=2048, D=2048, n_heads=16):
        self.B = B
        self.S = S
        self.D = D
        self.n_heads = n_heads
        self.Dh = 128
        self.DHT = n_heads * self.Dh       # W_qkv section stride (q/k/v)
        self.HPC = n_heads // N_CORES      # heads per core (2)
        self.QC = 512                      # token chunk (matmul free dim)
        self.KT = D // 128                 # k-tiles over d_model
        self.NCH = S // self.QC            # token chunks per batch
        assert self.HPC == 2 and D % 128 == 0 and S % self.QC == 0


def build_kernel(cfg: Cfg):
    """Build the SPMD single-core program. Returns compiled nc."""
    B, S, D, QC, KT, NCH = cfg.B, cfg.S, cfg.D, cfg.QC, cfg.KT, cfg.NCH
    Dh = cfg.Dh
    NQT = QC // 128                      # 128-token subtiles per chunk
    inv_sqrt_dh = 1.0 / math.sqrt(Dh)

    nc = bacc.Bacc("TRN2", target_bir_lowering=False, debug=False,
                   num_devices=N_CORES)

    xT = nc.dram_tensor("xT", [D, B * S], BF16, kind="ExternalInput").ap()
    wqkvT = nc.dram_tensor("wqkvT", [D, 768], BF16, kind="ExternalInput").ap()
    woutT = nc.dram_tensor("woutT", [256, D], BF16, kind="ExternalInput").ap()
    ident = nc.dram_tensor("ident", [128, 128], BF16,
                           kind="ExternalInput").ap()
    tri = nc.dram_tensor("tri", [128, 128], BF16, kind="ExternalInput").ap()
    ones_col = nc.dram_tensor("ones_col", [128, 1], BF16,
                              kind="ExternalInput").ap()
    ones_row = nc.dram_tensor("ones_row", [1, 128], BF16,
                              kind="ExternalInput").ap()
    outT = nc.dram_tensor("outT", [D, B * S], BF16, kind="ExternalOutput").ap()

    with tile.TileContext(nc) as tc:
        with (
            tc.tile_pool(name="wpool", bufs=1) as wpool,
            tc.tile_pool(name="xpool", bufs=9) as xpool,
            tc.tile_pool(name="qkvpool", bufs=1) as qkvpool,
            tc.tile_pool(name="apool", bufs=1) as apool,
            tc.tile_pool(name="ppool", bufs=12) as ppool,
            tc.tile_pool(name="opool", bufs=4) as opool,
            tc.tile_pool(name="smallpool", bufs=4) as smallpool,
            # 3 x [128,1024] two-bank score tiles; also serves QKV,
            # rank-1 and out-proj psums (sequential on the PE timeline)
            tc.tile_pool(name="scoreps", bufs=3, space="PSUM") as scoreps,
            tc.tile_pool(name="attnps", bufs=1, space="PSUM") as attnps,
            tc.tile_pool(name="rps", bufs=1, space="PSUM") as rps,
        ):
            HKT = KT // 4

            def load_x_chunk(b, j):
                """Four quarter DMAs for one 512-token chunk of x^T."""
                col0 = b * S + j * QC
                halves = []
                for hh in range(4):
                    t = xpool.tile([128, HKT * QC], BF16, tag="xt", name="xt")
                    src = xT[hh * HKT * 128:(hh + 1) * HKT * 128,
                             col0:col0 + QC]
                    nc.sync.dma_start(
                        t[:].rearrange("p (k c) -> p k c", k=HKT),
                        src.rearrange("(k p) c -> p k c", p=128))
                    halves.append(t)
                return halves

            # ---- static weights / constants, interleaved so the first
            # QKV matmuls (need w k-tile 0 + x chunk 0) start ASAP ----
            w_tiles = []
            for k in range(4):
                t = wpool.tile([128, 768], BF16, tag=f"w{k}", name=f"w{k}")
                nc.sync.dma_start(t[:], wqkvT[k * 128:(k + 1) * 128, :])
                w_tiles.append(t)
            first_chunk = load_x_chunk(0, 0)
            for k in range(4, KT):
                t = wpool.tile([128, 768], BF16, tag=f"w{k}", name=f"w{k}")
                nc.sync.dma_start(t[:], wqkvT[k * 128:(k + 1) * 128, :])
                w_tiles.append(t)
            onec_t = wpool.tile([128, 1], BF16, tag="onec", name="onec")
            nc.sync.dma_start(onec_t[:], ones_col[:])
            oner_t = wpool.tile([1, 128], BF16, tag="oner", name="oner")
            nc.sync.dma_start(oner_t[:], ones_row[:])
            ident_t = wpool.tile([128, 128], BF16, tag="ident", name="ident")
            nc.sync.dma_start(ident_t[:], ident[:])
            tri_t = wpool.tile([128, 128], BF16, tag="tri", name="tri")
            nc.sync.dma_start(tri_t[:], tri[:])
            wo_tiles = []
            for hh in range(2):
                t = wpool.tile([128, D], BF16, tag=f"wo{hh}", name=f"wo{hh}")
                nc.sync.dma_start(t[:], woutT[hh * 128:(hh + 1) * 128, :])
                wo_tiles.append(t)

            for b in range(B):
                # ---- persistent per-batch QKV / A tiles ----
                # comps: 0=Q_h0 1=K_h0 2=Q_h1 3=K_h1 (dh-major [128, S])
                qk_sb = [qkvpool.tile([128, S], BF16, tag=f"qk{c}", name=f"qk{c}")
                         for c in range(4)]
                # V token-major: tile per 128 tokens, [128, 256] (2 heads)
                v_sb = [qkvpool.tile([128, 256], BF16, tag=f"v{t}", name=f"v{t}")
                        for t in range(S // 128)]
                # A^T per head [128, S]
                a_sb = [apool.tile([128, S], BF16, tag=f"a{h}", name=f"a{h}")
                        for h in range(2)]

                def emit_qkv_chunk(j):
                    if b == 0 and j == 0:
                        halves = first_chunk
                    else:
                        halves = load_x_chunk(b, j)

                    def xt_sl(k, f0, f1):
                        t = halves[k // HKT]
                        kk = k % HKT
                        return t[:, kk * QC + f0: kk * QC + f1]

                    # Q^T / K^T for both heads (copies on ScalarE; they
                    # complete inside this chunk's own PE window)
                    for c in range(4):
                        ps = scoreps.tile([128, QC], F32, tag="ps", name="ps")
                        for k in range(KT):
                            nc.tensor.matmul(
                                ps[:],
                                (w_tiles[k][:, c * 128:(c + 1) * 128]),
                                (xt_sl(k, 0, QC)),
                                start=(k == 0), stop=(k == KT - 1))
                        nc.scalar.copy(
                            qk_sb[c][:, j * QC:(j + 1) * QC], ps[:])
                    # V token-major (both heads side by side)
                    for sub in range(NQT):
                        ps = scoreps.tile([128, 256], F32, tag="ps", name="ps")
                        for k in range(KT):
                            nc.tensor.matmul(
                                ps[:],
                                (xt_sl(k, sub * 128, (sub + 1) * 128)),
                                (w_tiles[k][:, 512:768]),
                                start=(k == 0), stop=(k == KT - 1))
                        nc.scalar.copy(v_sb[j * NQT + sub][:], ps[:])

                def emit_attn_block(j, h):
                    # scores in PAIRS of k-tiles -> one exp per pair; AV/r
                    # lag by GSKEW pairs so the PSUM->exp latency stays off
                    # the PE's in-order path.
                    GSKEW = 2
                    n_kt = (j + 1) * QC // 128
                    n_g = n_kt // 2
                    qT = qk_sb[2 * h]
                    kTl = qk_sb[2 * h + 1]
                    attn = attnps.tile([128, QC], F32, tag="attn",
                                       name="attn")
                    r = rps.tile([1, QC], F32, tag="r", name="r")
                    groups = {}

                    def emit_scores_group(g):
                        s_ps = scoreps.tile([128, 2 * QC], F32, tag="ps",
                                            name="ps")
                        f_avs = []
                        for i, kt in enumerate((2 * g, 2 * g + 1)):
                            rel = kt * 128 - j * QC
                            f_av = max(rel, 0)
                            diag = rel >= 0
                            base = i * QC
                            nc.tensor.matmul(
                                s_ps[:, base + f_av:base + QC],
                                kTl[:, kt * 128:(kt + 1) * 128],
                                qT[:, j * QC + f_av:(j + 1) * QC],
                                start=True, stop=not diag)
                            if diag:
                                # add -1e9 to the k > q triangle (I.T @ tri)
                                # inside the accumulation group
                                nc.tensor.matmul(
                                    s_ps[:, base + rel:base + rel + 128],
                                    ident_t[:], tri_t[:],
                                    start=False, stop=True)
                            f_avs.append(f_av)
                        p_sb = ppool.tile([128, 2 * QC], BF16, tag="p",
                                          name="p")
                        # one exp for the pair; leading trim only (the
                        # skipped region is never read downstream)
                        f0 = f_avs[0]
                        nc.scalar.activation(
                            p_sb[:, f0:], s_ps[:, f0:],
                            mybir.ActivationFunctionType.Exp,
                            scale=inv_sqrt_dh)
                        groups[g] = (p_sb, f_avs)

                    def emit_av_group(g):
                        p_sb, f_avs = groups.pop(g)
                        for i, kt in enumerate((2 * g, 2 * g + 1)):
                            f_av = f_avs[i]
                            base = i * QC
                            nc.tensor.matmul(
                                attn[:, f_av:],
                                v_sb[kt][:, h * 128:(h + 1) * 128],
                                p_sb[:, base + f_av:base + QC],
                                start=(kt == 0), stop=(kt == n_kt - 1))
                            nc.tensor.matmul(
                                r[:, f_av:], onec_t[:],
                                p_sb[:, base + f_av:base + QC],
                                start=(kt == 0), stop=(kt == n_kt - 1))

                    for g in range(n_g):
                        emit_scores_group(g)
                        if g >= GSKEW:
                            emit_av_group(g - GSKEW)
                    for g in range(max(0, n_g - GSKEW), n_g):
                        emit_av_group(g)
                    # drain attn psum to SBUF now (frees the single attn
                    # bank) and launch the reciprocal (DVE); both consumed
                    # one block later
                    attn_sb = ppool.tile([128, QC], F32R, tag="p", name="p")
                    nc.vector.tensor_copy(attn_sb[:], attn[:])
                    recip = smallpool.tile([1, QC], F32, tag="recip",
                                           name="recip")
                    nc.vector.reciprocal_approx_fast(recip[:], r[:])
                    recip_b = smallpool.tile([1, QC], BF16, tag="recipb",
                                             name="recipb")
                    nc.vector.tensor_copy(recip_b[:], recip[:])
                    return (j, h, attn_sb, recip_b)

                def emit_finalize(blk):
                    j, h, attn_sb, recip_b = blk
                    rb_ps = scoreps.tile([128, QC], F32, tag="ps", name="ps")
                    nc.tensor.matmul(rb_ps[:], oner_t[:], recip_b[:],
                                     start=True, stop=True)
                    rb_sb = ppool.tile([128, QC], F32R, tag="p", name="p")
                    nc.vector.tensor_copy(rb_sb[:], rb_ps[:])
                    nc.vector.tensor_mul(
                        a_sb[h][:, j * QC:(j + 1) * QC], attn_sb[:],
                        rb_sb[:])

                def emit_outproj(j):
                    # partial over this core's 256 head-features; drains on
                    # DVE (ACT is saturated by exp), stores on the idle
                    # gpsimd queue.
                    col0 = b * S + j * QC
                    for m in range(D // 128):
                        ps = scoreps.tile([128, QC], F32, tag="ps", name="ps")
                        for h in range(2):
                            nc.tensor.matmul(
                                ps[:],
                                wo_tiles[h][:, m * 128:(m + 1) * 128],
                                a_sb[h][:, j * QC:(j + 1) * QC],
                                start=(h == 0), stop=(h == 1))
                        o_sb = opool.tile([128, QC], BF16, tag="o", name="o")
                        nc.vector.tensor_copy(o_sb[:], ps[:])
                        nc.gpsimd.dma_start(
                            outT[m * 128:(m + 1) * 128, col0:col0 + QC],
                            o_sb[:])

                blocks = []
                for j in range(NCH):
                    emit_qkv_chunk(j)
                    for h in range(2):
                        blocks.append(emit_attn_block(j, h))
                        n = len(blocks)
                        if n >= 2:
                            emit_finalize(blocks[n - 2])
                        if n >= 3 and blocks[n - 3][1] == 1:
                            emit_outproj(blocks[n - 3][0])
                emit_finalize(blocks[-1])
                emit_outproj(blocks[-1][0])

    nc.compile()
    return nc


def make_inputs(cfg: Cfg, x, W_qkv, W_out):
    """Host-side sharding: returns in_maps (list of 8 dicts)."""
    B, S, D = cfg.B, cfg.S, cfg.D
    Dh = cfg.Dh
    xTa = np.ascontiguousarray(
        x.reshape(B * S, D).T).astype(ml_dtypes.bfloat16)  # [D, B*S]

    p = np.arange(128)[:, None]
    c = np.arange(128)[None, :]
    tri = np.where(p > c, -1e9, 0.0).astype(ml_dtypes.bfloat16)
    identm = np.eye(128, dtype=ml_dtypes.bfloat16)
    ones_col = np.ones((128, 1), dtype=ml_dtypes.bfloat16)
    ones_row = np.ones((1, 128), dtype=ml_dtypes.bfloat16)

    in_maps = []
    DHT = cfg.DHT
    for cidx in range(N_CORES):
        h0 = cfg.HPC * cidx
        wq = np.empty((D, 768), dtype=np.float32)          # [D, cols]
        for i, h in enumerate((h0, h0 + 1)):
            wq[:, (2 * i) * 128:(2 * i) * 128 + 128] = \
                W_qkv[0 * DHT + h * Dh: 0 * DHT + h * Dh + Dh, :].T   # Q_h
            wq[:, (2 * i + 1) * 128:(2 * i + 1) * 128 + 128] = \
                W_qkv[1 * DHT + h * Dh: 1 * DHT + h * Dh + Dh, :].T   # K_h
            wq[:, 512 + i * 128: 512 + (i + 1) * 128] = \
                W_qkv[2 * DHT + h * Dh: 2 * DHT + h * Dh + Dh, :].T   # V_h
        wo = np.ascontiguousarray(
            W_out[:, h0 * Dh:(h0 + cfg.HPC) * Dh].T).astype(
                ml_dtypes.bfloat16)
        in_maps.append({
            "xT": xTa,
            "wqkvT": wq.astype(ml_dtypes.bfloat16),
            "woutT": wo,
            "ident": identm,
            "tri": tri,
            "ones_col": ones_col,
            "ones_row": ones_row,
        })
    return in_maps


_CACHED = {}


def kernel(x, W_qkv, W_out, mask=None, **_ignored):
    cfg = Cfg(B=x.shape[0], S=x.shape[1], D=x.shape[2],
              n_heads=W_qkv.shape[0] // 384)
    key = (cfg.B, cfg.S, cfg.D)
    if key not in _CACHED:
        _CACHED[key] = build_kernel(cfg)
    nc = _CACHED[key]
    in_maps = make_inputs(cfg, np.asarray(x), np.asarray(W_qkv),
                          np.asarray(W_out))
    res = run_bass_kernel_spmd(nc, in_maps, list(range(N_CORES)))
    acc = res.results[0]["outT"].astype(np.float32)
    for c in range(1, N_CORES):
        acc = acc + res.results[c]["outT"].astype(np.float32)
    out = acc.T.reshape(cfg.B, cfg.S, cfg.D)
    return np.ascontiguousarray(out)


# revision 11
# speedup vs baseline: 1.2433x; 1.0544x over previous
"""Multi-head causal self-attention (B=2, S=2048, D=2048, H=16) on 8 trn2
NeuronCores.

Sharding: tensor-parallel over heads. Core c owns heads {2c, 2c+1}:
  - QKV projection for its 2 heads (contraction over the full d_model),
  - causal attention for its 2 heads,
  - partial output projection  O_c = A_c @ W_out[:, c*256:(c+1)*256].T
Host sums the 8 partial outputs (the "all-reduce after out_proj" of the
TP scheme, done on host since the full output is assembled there anyway).

All on-device compute is laid out "feature-major" (transposed) so no
transposes are ever needed. Everything is bf16 except the PSUM (f32)
and the normalizer reciprocal: bf16 keeps every matmul at full PE rate
regardless of free-dim, enables fast weight load on all stationaries,
and halves DMA/SBUF traffic.

The attention inner loop is co-limited by the PE and the ACT engine
(exp costs (N+352)/1.2 ns per instruction), so scores are computed in
PAIRS of 128-key tiles into a two-bank [128, 1024] PSUM tile and a
single exp covers both: 573 ns/k-tile instead of 720. Three such score
tiles rotate (6 banks); attn and the normalizer each hold one bank and
are drained to SBUF at block end so bufs=1 suffices. The causal mask is
applied inside the scores accumulation group via one extra 128-row
matmul (adds -1e9 to the k > q triangle), keeping the exp -> AV chain
entirely off the vector engine.

Structure per batch (attention interleaved with QKV by chunk):
  for j: QKV(chunk j) -> attn(j,h0) -> fin(prev) -> attn(j,h1)
         -> fin(j,h0) -> outproj(j-1)
AV lags scores by 2 groups (4 k-tiles), normalization lags one block,
out-proj two blocks: every PE wait target is produced ~2.5 us earlier.
Output stores ride the gpsimd queue so input prefetch is never stuck
behind stores on the sync queue.
"""

import math

import ml_dtypes
import numpy as np

import concourse.bass as bass
import concourse.tile as tile
from concourse import bacc, mybir
from concourse.bass_utils import run_bass_kernel_spmd

F32 = mybir.dt.float32
F32R = mybir.dt.float32r
BF16 = mybir.dt.bfloat16

N_CORES = 8


class Cfg:
    def __init__(self, B=2, S=2048, D=2048, n_heads=16):
        self.B = B
        self.S = S
        self.D = D
        self.n_heads = n_heads
        self.Dh = 128
        self.DHT = n_heads * self.Dh       # W_qkv section stride (q/k/v)
        self.HPC = n_heads // N_CORES      # heads per core (2)
        self.QC = 512                      # token chunk (matmul free dim)
        self.KT = D // 128                 # k-tiles over d_model
        self.NCH = S // self.QC            # token chunks per batch
        assert self.HPC == 2 and D % 128 == 0 and S % self.QC == 0


def build_kernel(cfg: Cfg):
    """Build the SPMD single-core program. Returns compiled nc."""
    B, S, D, QC, KT, NCH = cfg.B, cfg.S, cfg.D, cfg.QC, cfg.KT, cfg.NCH
    Dh = cfg.Dh
    NQT = QC // 128                      # 128-token subtiles per chunk
    inv_sqrt_dh = 1.0 / math.sqrt(Dh)

    nc = bacc.Bacc("TRN2", target_bir_lowering=False, debug=False,
                   num_devices=N_CORES)

    xT = nc.dram_tensor("xT", [D, B * S], BF16, kind="ExternalInput").ap()
    wqkvT = nc.dram_tensor("wqkvT", [D, 768], BF16, kind="ExternalInput").ap()
    woutT = nc.dram_tensor("woutT", [256, D], BF16, kind="ExternalInput").ap()
    ident = nc.dram_tensor("ident", [128, 128], BF16,
                           kind="ExternalInput").ap()
    tri = nc.dram_tensor("tri", [128, 128], BF16, kind="ExternalInput").ap()
    ones_col = nc.dram_tensor("ones_col", [128, 1], BF16,
                              kind="ExternalInput").ap()
    ones_row = nc.dram_tensor("ones_row", [1, 128], BF16,
                              kind="ExternalInput").ap()
    outT = nc.dram_tensor("outT", [D, B * S], BF16, kind="ExternalOutput").ap()

    with tile.TileContext(nc) as tc:
        with (
            tc.tile_pool(name="wpool", bufs=1) as wpool,
            tc.tile_pool(name="xpool", bufs=9) as xpool,
            tc.tile_pool(name="qkvpool", bufs=1) as qkvpool,
            tc.tile_pool(name="apool", bufs=6) as apool,
            tc.tile_pool(name="ppool", bufs=12) as ppool,
            tc.tile_pool(name="opool", bufs=6) as opool,
            tc.tile_pool(name="smallpool", bufs=4) as smallpool,
            # 3 x [128,1024] two-bank score tiles; also serves QKV,
            # rank-1 and out-proj psums (sequential on the PE timeline)
            tc.tile_pool(name="scoreps", bufs=3, space="PSUM") as scoreps,
            tc.tile_pool(name="attnps", bufs=1, space="PSUM") as attnps,
            tc.tile_pool(name="rps", bufs=1, space="PSUM") as rps,
        ):
            HKT = KT // 4

            def load_x_chunk(b, j):
                """Four quarter DMAs for one 512-token chunk of x^T."""
                col0 = b * S + j * QC
                halves = []
                for hh in range(4):
                    t = xpool.tile([128, HKT * QC], BF16, tag="xt", name="xt")
                    src = xT[hh * HKT * 128:(hh + 1) * HKT * 128,
                             col0:col0 + QC]
                    nc.sync.dma_start(
                        t[:].rearrange("p (k c) -> p k c", k=HKT),
                        src.rearrange("(k p) c -> p k c", p=128))
                    halves.append(t)
                return halves

            # ---- static weights / constants, interleaved with the first
            # x chunk in exactly the k-loop consumption order (w k-quad,
            # then the x quarter that pairs with it) so the first QKV
            # psum group never waits on a later DMA ----
            w_tiles = [None] * KT
            first_chunk = []
            col0 = 0
            for hh in range(4):
                for k in range(hh * HKT, (hh + 1) * HKT):
                    t = wpool.tile([128, 768], BF16, tag=f"w{k}",
                                   name=f"w{k}")
                    nc.sync.dma_start(t[:], wqkvT[k * 128:(k + 1) * 128, :])
                    w_tiles[k] = t
                t = xpool.tile([128, HKT * QC], BF16, tag="xt", name="xt")
                src = xT[hh * HKT * 128:(hh + 1) * HKT * 128, col0:col0 + QC]
                nc.sync.dma_start(
                    t[:].rearrange("p (k c) -> p k c", k=HKT),
                    src.rearrange("(k p) c -> p k c", p=128))
                first_chunk.append(t)
            onec_t = wpool.tile([128, 1], BF16, tag="onec", name="onec")
            nc.sync.dma_start(onec_t[:], ones_col[:])
            oner_t = wpool.tile([1, 128], BF16, tag="oner", name="oner")
            nc.sync.dma_start(oner_t[:], ones_row[:])
            ident_t = wpool.tile([128, 128], BF16, tag="ident", name="ident")
            nc.sync.dma_start(ident_t[:], ident[:])
            tri_t = wpool.tile([128, 128], BF16, tag="tri", name="tri")
            nc.sync.dma_start(tri_t[:], tri[:])
            wo_tiles = []
            for hh in range(2):
                t = wpool.tile([128, D], BF16, tag=f"wo{hh}", name=f"wo{hh}")
                nc.sync.dma_start(t[:], woutT[hh * 128:(hh + 1) * 128, :])
                wo_tiles.append(t)

            for b in range(B):
                # ---- persistent per-batch QKV / A tiles ----
                # comps: 0=Q_h0 1=K_h0 2=Q_h1 3=K_h1 (dh-major [128, S])
                qk_sb = [qkvpool.tile([128, S], BF16, tag=f"qk{c}", name=f"qk{c}")
                         for c in range(4)]
                # V token-major: tile per 128 tokens, [128, 256] (2 heads)
                v_sb = [qkvpool.tile([128, 256], BF16, tag=f"v{t}", name=f"v{t}")
                        for t in range(S // 128)]
                # A^T per (chunk, head) [128, QC] — per-chunk tiles so
                # out-proj(j) depends only on its own chunk's writes (a
                # whole-S tile would serialize on the NEXT chunk's mul
                # via tile-granularity dependency tracking)
                a_sb = {}

                def emit_qkv_chunk(j):
                    if b == 0 and j == 0:
                        halves = first_chunk
                    else:
                        halves = load_x_chunk(b, j)

                    def xt_sl(k, f0, f1):
                        t = halves[k // HKT]
                        kk = k % HKT
                        return t[:, kk * QC + f0: kk * QC + f1]

                    # Q^T / K^T for both heads (copies on DVE: the ACT
                    # engine is reserved for exp, which must never fall
                    # behind the attention inner loop)
                    for c in range(4):
                        ps = scoreps.tile([128, QC], F32, tag="ps", name="ps")
                        for k in range(KT):
                            nc.tensor.matmul(
                                ps[:],
                                (w_tiles[k][:, c * 128:(c + 1) * 128]),
                                (xt_sl(k, 0, QC)),
                                start=(k == 0), stop=(k == KT - 1))
                        nc.vector.tensor_copy(
                            qk_sb[c][:, j * QC:(j + 1) * QC], ps[:])
                    # V token-major (both heads side by side)
                    for sub in range(NQT):
                        ps = scoreps.tile([128, 256], F32, tag="ps", name="ps")
                        for k in range(KT):
                            nc.tensor.matmul(
                                ps[:],
                                (xt_sl(k, sub * 128, (sub + 1) * 128)),
                                (w_tiles[k][:, 512:768]),
                                start=(k == 0), stop=(k == KT - 1))
                        nc.vector.tensor_copy(v_sb[j * NQT + sub][:], ps[:])

                def emit_attn_block(j, h):
                    # scores in PAIRS of k-tiles -> one exp per pair; AV/r
                    # lag by GSKEW pairs so the PSUM->exp latency stays off
                    # the PE's in-order path.
                    GSKEW = 3
                    n_kt = (j + 1) * QC // 128
                    n_g = n_kt // 2
                    qT = qk_sb[2 * h]
                    kTl = qk_sb[2 * h + 1]
                    attn = attnps.tile([128, QC], F32, tag="attn",
                                       name="attn")
                    r = rps.tile([1, QC], F32, tag="r", name="r")
                    groups = {}

                    def emit_scores_group(g):
                        s_ps = scoreps.tile([128, 2 * QC], F32, tag="ps",
                                            name="ps")
                        f_avs = []
                        for i, kt in enumerate((2 * g, 2 * g + 1)):
                            rel = kt * 128 - j * QC
                            f_av = max(rel, 0)
                            diag = rel >= 0
                            base = i * QC
                            nc.tensor.matmul(
                                s_ps[:, base + f_av:base + QC],
                                kTl[:, kt * 128:(kt + 1) * 128],
                                qT[:, j * QC + f_av:(j + 1) * QC],
                                start=True, stop=not diag)
                            if diag:
                                # add -1e9 to the k > q triangle (I.T @ tri)
                                # inside the accumulation group
                                nc.tensor.matmul(
                                    s_ps[:, base + rel:base + rel + 128],
                                    ident_t[:], tri_t[:],
                                    start=False, stop=True)
                            f_avs.append(f_av)
                        p_sb = ppool.tile([128, 2 * QC], BF16, tag="p",
                                          name="p")
                        # one exp for the pair; leading trim only (the
                        # skipped region is never read downstream)
                        f0 = f_avs[0]
                        nc.scalar.activation(
                            p_sb[:, f0:], s_ps[:, f0:],
                            mybir.ActivationFunctionType.Exp,
                            scale=inv_sqrt_dh)
                        groups[g] = (p_sb, f_avs)

                    def emit_av_group(g):
                        p_sb, f_avs = groups.pop(g)
                        for i, kt in enumerate((2 * g, 2 * g + 1)):
                            f_av = f_avs[i]
                            base = i * QC
                            nc.tensor.matmul(
                                attn[:, f_av:],
                                v_sb[kt][:, h * 128:(h + 1) * 128],
                                p_sb[:, base + f_av:base + QC],
                                start=(kt == 0), stop=(kt == n_kt - 1))
                            nc.tensor.matmul(
                                r[:, f_av:], onec_t[:],
                                p_sb[:, base + f_av:base + QC],
                                start=(kt == 0), stop=(kt == n_kt - 1))

                    for g in range(n_g):
                        emit_scores_group(g)
                        if g >= GSKEW:
                            emit_av_group(g - GSKEW)
                    for g in range(max(0, n_g - GSKEW), n_g):
                        emit_av_group(g)
                    # drain attn psum to SBUF now (frees the single attn
                    # bank) and launch the reciprocal (DVE); both consumed
                    # one block later
                    attn_sb = ppool.tile([128, QC], F32R, tag="p", name="p")
                    nc.vector.tensor_copy(attn_sb[:], attn[:])
                    recip = smallpool.tile([1, QC], F32, tag="recip",
                                           name="recip")
                    nc.vector.reciprocal_approx_fast(recip[:], r[:])
                    recip_b = smallpool.tile([1, QC], BF16, tag="recipb",
                                             name="recipb")
                    nc.vector.tensor_copy(recip_b[:], recip[:])
                    return (j, h, attn_sb, recip_b)

                def emit_finalize(blk):
                    j, h, attn_sb, recip_b = blk
                    rb_ps = scoreps.tile([128, QC], F32, tag="ps", name="ps")
                    nc.tensor.matmul(rb_ps[:], oner_t[:], recip_b[:],
                                     start=True, stop=True)
                    rb_sb = ppool.tile([128, QC], F32R, tag="p", name="p")
                    nc.vector.tensor_copy(rb_sb[:], rb_ps[:])
                    at = apool.tile([128, QC], BF16, tag="a", name="a")
                    nc.vector.tensor_mul(at[:], attn_sb[:], rb_sb[:])
                    a_sb[(j, h)] = at

                def emit_outproj(j, last=False):
                    # partial over this core's 256 head-features; drains on
                    # DVE (ACT is saturated by exp — except on the final
                    # chunk, where no exps remain and alternating halves
                    # the drain tail); stores alternate between the two
                    # otherwise-idle DMA queues.
                    col0 = b * S + j * QC
                    for m in range(D // 128):
                        ps = scoreps.tile([128, QC], F32, tag="ps", name="ps")
                        for h in range(2):
                            nc.tensor.matmul(
                                ps[:],
                                wo_tiles[h][:, m * 128:(m + 1) * 128],
                                a_sb[(j, h)][:],
                                start=(h == 0), stop=(h == 1))
                        o_sb = opool.tile([128, QC], BF16, tag="o", name="o")
                        if last and m % 2 == 1:
                            nc.scalar.copy(o_sb[:], ps[:])
                        else:
                            nc.vector.tensor_copy(o_sb[:], ps[:])
                        q = nc.gpsimd if m % 2 == 0 else nc.sync
                        q.dma_start(
                            outT[m * 128:(m + 1) * 128, col0:col0 + QC],
                            o_sb[:])

                blocks = []
                for j in range(NCH):
                    emit_qkv_chunk(j)
                    for h in range(2):
                        blocks.append(emit_attn_block(j, h))
                        n = len(blocks)
                        if n >= 2:
                            emit_finalize(blocks[n - 2])
                        if n >= 3 and blocks[n - 3][1] == 1:
                            emit_outproj(blocks[n - 3][0])
                emit_finalize(blocks[-1])
                emit_outproj(blocks[-1][0], last=(b == B - 1))

    nc.compile()
    return nc


def make_inputs(cfg: Cfg, x, W_qkv, W_out):
    """Host-side sharding: returns in_maps (list of 8 dicts)."""
    B, S, D = cfg.B, cfg.S, cfg.D
    Dh = cfg.Dh
    xTa = np.ascontiguousarray(
        x.reshape(B * S, D).T).astype(ml_dtypes.bfloat16)  # [D, B*S]

    p = np.arange(128)[:, None]
    c = np.arange(128)[None, :]
    tri = np.where(p > c, -1e9, 0.0).astype(ml_dtypes.bfloat16)
    identm = np.eye(128, dtype=ml_dtypes.bfloat16)
    ones_col = np.ones((128, 1), dtype=ml_dtypes.bfloat16)
    ones_row = np.ones((1, 128), dtype=ml_dtypes.bfloat16)

    in_maps = []
    DHT = cfg.DHT
    for cidx in range(N_CORES):
        h0 = cfg.HPC * cidx
        wq = np.empty((D, 768), dtype=np.float32)          # [D, cols]
        for i, h in enumerate((h0, h0 + 1)):
            wq[:, (2 * i) * 128:(2 * i) * 128 + 128] = \
                W_qkv[0 * DHT + h * Dh: 0 * DHT + h * Dh + Dh, :].T   # Q_h
            wq[:, (2 * i + 1) * 128:(2 * i + 1) * 128 + 128] = \
                W_qkv[1 * DHT + h * Dh: 1 * DHT + h * Dh + Dh, :].T   # K_h
            wq[:, 512 + i * 128: 512 + (i + 1) * 128] = \
                W_qkv[2 * DHT + h * Dh: 2 * DHT + h * Dh + Dh, :].T   # V_h
        wo = np.ascontiguousarray(
            W_out[:, h0 * Dh:(h0 + cfg.HPC) * Dh].T).astype(
                ml_dtypes.bfloat16)
        in_maps.append({
            "xT": xTa,
            "wqkvT": wq.astype(ml_dtypes.bfloat16),
            "woutT": wo,
            "ident": identm,
            "tri": tri,
            "ones_col": ones_col,
            "ones_row": ones_row,
        })
    return in_maps


_CACHED = {}


def kernel(x, W_qkv, W_out, mask=None, **_ignored):
    cfg = Cfg(B=x.shape[0], S=x.shape[1], D=x.shape[2],
              n_heads=W_qkv.shape[0] // 384)
    key = (cfg.B, cfg.S, cfg.D)
    if key not in _CACHED:
        _CACHED[key] = build_kernel(cfg)
    nc = _CACHED[key]
    in_maps = make_inputs(cfg, np.asarray(x), np.asarray(W_qkv),
                          np.asarray(W_out))
    res = run_bass_kernel_spmd(nc, in_maps, list(range(N_CORES)))
    acc = res.results[0]["outT"].astype(np.float32)
    for c in range(1, N_CORES):
        acc = acc + res.results[c]["outT"].astype(np.float32)
    out = acc.T.reshape(cfg.B, cfg.S, cfg.D)
    return np.ascontiguousarray(out)
